# revision 1
# baseline (speedup 1.0000x reference)
"""Single-head causal self-attention (B=8, T=2048, D=512, H=64), data-parallel
over batch across 8 NeuronCores. Self-contained: builds a Bass/Tile kernel and
runs it via run_bass_kernel_spmd.

Per-core layout (batch element b = core id):
  - x [2048, 512] is PE-transposed to xT (d on partitions, f32r)
  - kT/qT [64, 2048] and v [2048, 64] projections in f32r; Wq, bq pre-scaled
    by H^-0.5 on the host; v is augmented with a ones column so the PV matmul
    also accumulates the softmax denominator
  - attention runs in S^T layout per 512-wide i-block: S^T = kT_chunk^T @ qT,
    exp on ACT (PSUM->SBUF, two j-tiles per instruction), multiplicative
    causal masks on the 4 diagonal j-tiles, PV matmul accumulates [65, 512]
  - epilogue: PE transpose of [65, 128] chunks, reciprocal * row, + bv
    (softmax rows sum to 1, so the v bias folds into the output)
"""

import sys

for _p in ("/root/.axon_site/_ro/trn_rl_repo", "/opt/trn_rl_repo"):
    if _p not in sys.path:
        sys.path.append(_p)

import numpy as np
import concourse.bass as bass
import concourse.bacc as bacc
import concourse.tile as tile
from concourse import mybir
from concourse.bass_utils import run_bass_kernel_spmd
from concourse.masks import make_identity

F32 = mybir.dt.float32
F32R = mybir.dt.float32r

B, T, D, H = 8, 2048, 512, 64
NT = T // 128   # 16 t-tiles
ND = D // 128   # 4 d-chunks
NIB = T // 512  # 4 i-blocks
EXP = mybir.ActivationFunctionType.Exp


def build_body(nc, tc, ctx, dram, repeat=1):
    x_d, w_d, bkq_d, bv_d, out_d = dram

    persist = ctx.enter_context(tc.tile_pool(name="persist", bufs=1))
    epool = ctx.enter_context(tc.tile_pool(name="epool", bufs=6))
    otspool = ctx.enter_context(tc.tile_pool(name="otspool", bufs=3))
    opool = ctx.enter_context(tc.tile_pool(name="opool", bufs=3))
    rpool = ctx.enter_context(tc.tile_pool(name="rpool", bufs=3))
    pspool = ctx.enter_context(tc.tile_pool(name="ps", bufs=2, space="PSUM"))
    ps2pool = ctx.enter_context(tc.tile_pool(name="ps2", bufs=2, space="PSUM"))
    otppool = ctx.enter_context(tc.tile_pool(name="otp", bufs=2, space="PSUM"))

    # --- constants ---
    ident = persist.tile([128, 128], F32)
    make_identity(nc, ident[:])

    bkq_sb = persist.tile([64, 2], F32)
    bv_row = persist.tile([1, 64], F32)
    bvB = persist.tile([128, 64], F32)
    nc.gpsimd.dma_start(bkq_sb[:], bkq_d[:])
    nc.gpsimd.dma_start(bv_row[:], bv_d[:])
    nc.gpsimd.partition_broadcast(bvB[:], bv_row[:])

    # weights -> f32r (packed [ND, 128, 3*64]: k | q | v along last axis)
    wstage = persist.tile([128, ND, 3 * H], F32)
    nc.gpsimd.dma_start(wstage[:], w_d.rearrange("a p h -> p a h"))
    w_r = persist.tile([128, ND, 3 * H], F32R)
    nc.vector.tensor_copy(w_r[:], wstage[:])

    ones_col = persist.tile([128, 1], F32)
    nc.vector.memset(ones_col[:], 1.0)

    # persistent activations
    x_all = persist.tile([128, NT, D], F32)
    xT = persist.tile([128, ND, T], F32R)     # x transposed, d on partitions
    kT = persist.tile([64, T], F32R)
    qT = persist.tile([64, T], F32R)
    vTs = persist.tile([64, T], F32)
    v_aug = persist.tile([128, NT, 65], F32R)  # v rows + ones column
    o_all = persist.tile([128, NT, 64], F32)

    for rep in range(repeat):
        for jt in range(NT):
            nc.vector.tensor_copy(v_aug[:, jt, 64:65], ones_col[:])

        # x in: staged DMAs, small first so transposes start early
        t0 = 0
        for gi, ntile in enumerate((2, 2, 4, 4, 4)):
            eng = nc.sync if gi % 2 == 0 else nc.scalar
            eng.dma_start(
                x_all[:, t0:t0 + ntile, :],
                x_d[128 * t0:128 * (t0 + ntile), :].rearrange(
                    "(a p) d -> p a d", p=128),
            )
            t0 += ntile

        # per 512-wide t-chunk: transpose x, project k/q/v, build v_aug
        for tch in range(4):
            tsl = slice(tch * 512, (tch + 1) * 512)
            for dc in range(ND):
                tp = pspool.tile([128, 4, 128], F32, tag="ps")
                for q in range(4):
                    ti = 4 * tch + q
                    nc.tensor.transpose(
                        tp[:, q, :], x_all[:, ti, dc * 128:(dc + 1) * 128],
                        ident[:])
                nc.vector.tensor_copy(xT[:, dc, tsl], tp[:])

            k_ps = pspool.tile([64, 512], F32, tag="ps")
            for dc in range(ND):
                nc.tensor.matmul(k_ps[:], w_r[:, dc, 0:64], xT[:, dc, tsl],
                                 start=(dc == 0), stop=(dc == ND - 1))
            nc.vector.tensor_scalar_add(kT[:, tsl], k_ps[:], bkq_sb[:, 0:1])

            q_ps = pspool.tile([64, 512], F32, tag="ps")
            for dc in range(ND):
                nc.tensor.matmul(q_ps[:], w_r[:, dc, 64:128], xT[:, dc, tsl],
                                 start=(dc == 0), stop=(dc == ND - 1))
            nc.vector.tensor_scalar_add(qT[:, tsl], q_ps[:], bkq_sb[:, 1:2])

            v_ps = pspool.tile([64, 512], F32, tag="ps")
            for dc in range(ND):
                nc.tensor.matmul(v_ps[:], w_r[:, dc, 128:192], xT[:, dc, tsl],
                                 start=(dc == 0), stop=(dc == ND - 1))
            nc.vector.tensor_copy(vTs[:, tsl], v_ps[:])

            va_ps = pspool.tile([128, 4, 64], F32, tag="ps")
            for q in range(4):
                jt = 4 * tch + q
                nc.tensor.transpose(va_ps[:, q, :], vTs[:, jt * 128:(jt + 1) * 128],
                                    ident[0:64, 0:64])
            nc.vector.tensor_copy(v_aug[:, 4 * tch:4 * tch + 4, 0:64], va_ps[:])

        # --- attention per 512-wide i-block, S^T layout, j-tiles in pairs ---
        for bi in range(NIB):
            isl = slice(bi * 512, (bi + 1) * 512)
            njt = 4 * (bi + 1)
            ot_ps = otppool.tile([65, 512], F32, tag="ot")
            for jp in range(njt // 2):
                st2 = ps2pool.tile([128, 2, 512], F32, tag="ps2")
                for h in range(2):
                    jt = 2 * jp + h
                    nc.tensor.matmul(st2[:, h, :], kT[:, jt * 128:(jt + 1) * 128],
                                     qT[:, isl], start=True, stop=True)
                e2 = epool.tile([128, 2, 512], F32R, tag="e")
                nc.scalar.activation(e2[:], st2[:], EXP)
                for h in range(2):
                    jt = 2 * jp + h
                    if jt >= 4 * bi:
                        nc.gpsimd.affine_select(
                            out=e2[:, h, :], in_=e2[:, h, :],
                            compare_op=mybir.AluOpType.is_ge, fill=0.0,
                            base=-128 * (jt - 4 * bi),
                            pattern=[[1, 512]], channel_multiplier=-1)
                    nc.tensor.matmul(ot_ps[:], v_aug[:, jt, :], e2[:, h, :],
                                     start=(jt == 0), stop=(jt == njt - 1))

            ots = otspool.tile([65, 512], F32, tag="ots")
            nc.vector.tensor_copy(ots[:], ot_ps[:])
            for c in range(4):
                o_ps = pspool.tile([128, 65], F32, tag="ps")
                nc.tensor.transpose(o_ps[:], ots[:, c * 128:(c + 1) * 128],
                                    ident[0:65, 0:65])
                rec = rpool.tile([128, 1], F32, tag="r")
                nc.vector.reciprocal(rec[:], o_ps[:, 64:65])
                it = bi * 4 + c
                nc.vector.tensor_scalar_mul(o_all[:, it, :], o_ps[:, 0:64], rec[:])
                nc.vector.tensor_add(o_all[:, it, :], o_all[:, it, :], bvB[:])

        # output DMA per i-block so the store drains while later blocks run
        for bi in range(NIB):
            nc.sync.dma_start(
                out_d[512 * bi:512 * (bi + 1), :].rearrange(
                    "(a p) h -> p a h", p=128),
                o_all[:, 4 * bi:4 * bi + 4, :])


def build_nc(repeat=1):
    nc = bacc.Bacc("TRN2", target_bir_lowering=False, debug=False, num_devices=8)
    x_d = nc.dram_tensor("x", [T, D], F32, kind="ExternalInput")
    w_d = nc.dram_tensor("w", [ND, 128, 3 * H], F32, kind="ExternalInput")
    bkq_d = nc.dram_tensor("bkq", [H, 2], F32, kind="ExternalInput")
    bv_d = nc.dram_tensor("bv", [1, H], F32, kind="ExternalInput")
    out_d = nc.dram_tensor("out", [T, H], F32, kind="ExternalOutput")
    dram = (x_d, w_d, bkq_d, bv_d, out_d)

    from contextlib import ExitStack
    with tile.TileContext(nc) as tc:
        with ExitStack() as ctx:
            build_body(nc, tc, ctx, dram, repeat=repeat)
    nc.compile()
    return nc


_NC_CACHE = {}


def _get_nc(repeat=1):
    if repeat not in _NC_CACHE:
        _NC_CACHE[repeat] = build_nc(repeat)
    return _NC_CACHE[repeat]


def make_in_maps(x, Wk, bk, Wq, bq, Wv, bv):
    scale = float(H) ** -0.5
    w = np.concatenate(
        [Wk.reshape(ND, 128, H), (Wq * scale).reshape(ND, 128, H),
         Wv.reshape(ND, 128, H)], axis=2)
    w = np.ascontiguousarray(w)
    bkq = np.ascontiguousarray(np.stack([bk, bq * scale], axis=1))
    bvr = np.ascontiguousarray(bv.reshape(1, H))
    return [
        {"x": np.ascontiguousarray(x[b]), "w": w, "bkq": bkq, "bv": bvr}
        for b in range(B)
    ]


def kernel(x, Wk, bk, Wq, bq, Wv, bv, _repeat=1):
    x = np.asarray(x, dtype=np.float32)
    Wk = np.asarray(Wk, dtype=np.float32)
    bk = np.asarray(bk, dtype=np.float32)
    Wq = np.asarray(Wq, dtype=np.float32)
    bq = np.asarray(bq, dtype=np.float32)
    Wv = np.asarray(Wv, dtype=np.float32)
    bv = np.asarray(bv, dtype=np.float32)

    nc = _get_nc(_repeat)
    in_maps = make_in_maps(x, Wk, bk, Wq, bq, Wv, bv)
    res = run_bass_kernel_spmd(nc, in_maps, core_ids=list(range(B)))
    out = np.stack([res.results[b]["out"] for b in range(B)], axis=0)
    return out.astype(np.float32)



# revision 2
# speedup vs baseline: 1.2198x; 1.2198x over previous
"""Single-head causal self-attention (B=8, T=2048, D=512, H=64), data-parallel
over batch across 8 NeuronCores — v6.

vs v5:
  - warmup: a stream of dummy matmuls keeps PE busy through the DMA lead-in
    so the p-state ramp completes before real work arrives
  - one shared 3-bank PSUM pool (bufs=2) for kq-projection and S^T slots;
    diag pieces packed [m0 0:512][m1 512:896][m3 896:1024][m2 1024:1280] ->
    one exact-trim exp of ap 1280; full tiles grouped in triples -> 13 exps
  - projections emitted interleaved with attention blocks (time order)
  - PV chains accumulate full tiles first, diag last (shrinks the tail)
  - w uploaded pre-transposed [128, ND, 192] -> contiguous 1-desc/partition
"""

import sys

for _p in ("/root/.axon_site/_ro/trn_rl_repo", "/opt/trn_rl_repo"):
    if _p not in sys.path:
        sys.path.append(_p)

import numpy as np
import ml_dtypes
import concourse.bass as bass
import concourse.bacc as bacc
import concourse.tile as tile
from concourse import mybir
from concourse.bass_utils import run_bass_kernel_spmd

F32 = mybir.dt.float32
BF16 = mybir.dt.bfloat16
EXP = mybir.ActivationFunctionType.Exp

B, T, D, H = 8, 2048, 512, 64
ND = D // 128   # 4 d-chunks
NT = T // 128   # 16 t/j-tiles
NB = T // 512   # 4 i-blocks

# diag piece offsets in the flat [128, 1536] slot (all in-bank, contiguous
# coverage [0:1280]): m0 [0:512], m1 [512:896], m3 [896:1024], m2 [1024:1280]
DIAG_OFF = [0, 512, 1024, 896]
DIAG_N = [512, 384, 256, 128]


def build_body(nc, tc, ctx, dram, repeat=1):
    x_d, w_d, b_d, out_d = dram

    persist = ctx.enter_context(tc.tile_pool(name="persist", bufs=1))
    epool = ctx.enter_context(tc.tile_pool(name="epool", bufs=8))
    rpool = ctx.enter_context(tc.tile_pool(name="rpool", bufs=2))
    big = ctx.enter_context(tc.tile_pool(name="big", bufs=2, space="PSUM"))
    psX = ctx.enter_context(tc.tile_pool(name="psX", bufs=1, space="PSUM"))
    psO = ctx.enter_context(tc.tile_pool(name="psO", bufs=1, space="PSUM"))

    # --- PE warmup: dummy matmuls keep PE busy through the DMA lead-in so
    # the p-state ramp completes before real matmuls arrive ---
    wu = persist.tile([1, 512], BF16)
    nc.vector.memset(wu[:], 0.0)
    for i in range(10):
        wu_ps = big.tile([16, 512], F32, tag="st", name=f"wu{i}")
        nc.tensor.matmul(wu_ps[:], wu[:, 0:16], wu[:], start=True, stop=True)

    # --- persistent activations ---
    xT = [persist.tile([128, ND, 512], BF16, name=f"xT{t}") for t in range(4)]
    k_sb = persist.tile([H, T], BF16)
    q_sb = persist.tile([H, T], BF16)
    vT = persist.tile([H, T], BF16)
    v_aug = persist.tile([128, NT, H + 1], BF16)
    o_sb = persist.tile([128, NT, H], BF16)

    # --- constants ---
    w_all = persist.tile([128, ND, 192], BF16)   # [:,:,0:128]=wkq, [:,:,128:192]=wv
    b_all = persist.tile([H, 2], F32)            # col 0 = bq*scale, col 1 = bv

    ident = persist.tile([H, H], BF16)
    nc.vector.memset(ident[:], 1.0)
    nc.gpsimd.affine_select(out=ident[:], in_=ident[:],
                            compare_op=mybir.AluOpType.is_equal, fill=0.0,
                            base=0, pattern=[[1, H]], channel_multiplier=-1)

    tri = persist.tile([128, 128], BF16)
    nc.vector.memset(tri[:], 1.0)
    nc.gpsimd.affine_select(out=tri[:], in_=tri[:],
                            compare_op=mybir.AluOpType.is_ge, fill=0.0,
                            base=0, pattern=[[1, 128]], channel_multiplier=-1)

    nc.vector.memset(v_aug[:, :, H:H + 1], 1.0)

    for _rep in range(repeat):
        if _rep == 0:
            nc.gpsimd.dma_start(w_all[:], w_d[:])
            nc.gpsimd.dma_start(b_all[:], b_d[:])
        # --- x in: one DMA transpose per 512-wide t-chunk, SP/ACT alternate ---
        for tch in range(4):
            tsl = slice(tch * 512, (tch + 1) * 512)
            eng = nc.sync if tch % 2 == 0 else nc.scalar
            eng.dma_start_transpose(xT[tch][:], x_d[tsl, :])

        e_full = {}   # (b, g) -> tile; g = jt//3
        e_diag = {}   # b -> tile
        st_diag = {}
        o_ps_all = {}

        def emit_proj(tch):
            tsl = slice(tch * 512, (tch + 1) * 512)
            kq_ps = big.tile([128, 512], F32, tag="st", name=f"kq{tch}")
            for dc in range(ND):
                nc.tensor.matmul(kq_ps[:], w_all[:, dc, 0:128],
                                 xT[tch][:, dc, :],
                                 start=(dc == 0), stop=(dc == ND - 1))
            nc.vector.tensor_scalar_add(q_sb[:, tsl], kq_ps[H:128, :],
                                        b_all[:, 0:1])
            nc.vector.tensor_copy(k_sb[:, tsl], kq_ps[0:H, :])

            v_ps = psX.tile([H, 512], F32, tag="px")
            for dc in range(ND):
                nc.tensor.matmul(v_ps[:], w_all[:, dc, 128:192],
                                 xT[tch][:, dc, :],
                                 start=(dc == 0), stop=(dc == ND - 1))
            nc.vector.tensor_scalar_add(vT[:, tsl], v_ps[:], b_all[:, 1:2])

            vt_ps = psX.tile([128, 4, H], BF16, tag="px")
            for q in range(4):
                jt = 4 * tch + q
                nc.tensor.transpose(vt_ps[:, q, :],
                                    vT[:, jt * 128:(jt + 1) * 128], ident[:])
            nc.vector.tensor_copy(v_aug[:, 4 * tch:4 * tch + 4, 0:H], vt_ps[:])

        def emit_sfull(b, g):
            """One slot holding full j-tiles 3g .. min(3g+3, 4b)-1."""
            isl = slice(b * 512, (b + 1) * 512)
            njt = min(3 * g + 3, 4 * b) - 3 * g
            st = big.tile([128, 1536], F32, tag="st", name=f"stf{b}_{g}")
            for h in range(njt):
                jt = 3 * g + h
                nc.tensor.matmul(st[:, 512 * h:512 * (h + 1)],
                                 k_sb[:, jt * 128:(jt + 1) * 128],
                                 q_sb[:, isl], start=True, stop=True)
            ef = epool.tile([128, 1536], BF16, tag="e", name=f"ef{b}_{g}")
            nc.scalar.activation(ef[:, 0:512 * njt], st[:, 0:512 * njt], EXP)
            e_full[(b, g)] = ef

        def emit_sdiag(b, sl=None):
            ms = range(4) if sl is None else ((0, 1, 3) if sl == 0 else (2,))
            if sl in (None, 0):
                st = big.tile([128, 1536], F32, tag="st", name=f"std{b}")
                ed = epool.tile([128, 1536], BF16, tag="e", name=f"ed{b}")
                e_diag[b] = ed
                st_diag[b] = st
            else:
                st = st_diag[b]
                ed = e_diag[b]
            for m in ms:
                off, n = DIAG_OFF[m], DIAG_N[m]
                jt = 4 * b + m
                nc.tensor.matmul(
                    st[:, off:off + n],
                    k_sb[:, jt * 128:(jt + 1) * 128],
                    q_sb[:, b * 512 + 128 * m:(b + 1) * 512],
                    start=True, stop=True)
            if sl is None:
                nc.scalar.activation(ed[:, 0:1280], st[:, 0:1280], EXP)
            elif sl == 0:
                nc.scalar.activation(ed[:, 0:1024], st[:, 0:1024], EXP)
            else:
                nc.scalar.activation(ed[:, 1024:1280], st[:, 1024:1280], EXP)
            for m in ms:
                off = DIAG_OFF[m]
                nc.gpsimd.affine_select(
                    out=ed[:, off:off + 128], in_=ed[:, off:off + 128],
                    compare_op=mybir.AluOpType.is_ge, fill=0.0,
                    base=0, pattern=[[1, 128]], channel_multiplier=-1)

        def emit_pv(b, m_loc):
            if m_loc == 0:
                o_ps_all[b] = psO.tile([128, 4, H + 1], F32, tag="o",
                                       name=f"o_ps{b}")
            o_ps = o_ps_all[b]
            it = 4 * b + m_loc
            # full tiles first (ready earlier), diag pieces last
            order = [jt for jt in range(4 * b)] + \
                    [4 * b + m for m in range(m_loc + 1)]
            for idx, jt in enumerate(order):
                if jt < 4 * b:
                    src = e_full[(b, jt // 3)]
                    col = 512 * (jt % 3) + 128 * m_loc
                else:
                    m = jt - 4 * b
                    src = e_diag[b]
                    col = DIAG_OFF[m] + 128 * (m_loc - m)
                nc.tensor.matmul(o_ps[:, m_loc, :],
                                 src[:, col:col + 128],
                                 v_aug[:, jt, :],
                                 start=(idx == 0), stop=(idx == len(order) - 1))

        def emit_epilogue(b):
            o_ps = o_ps_all[b]
            rec = rpool.tile([128, 4], F32, tag="r")
            nc.vector.reciprocal(rec[:], o_ps[:, :, H:H + 1].rearrange(
                "p a o -> p (a o)"))
            rec_b = rec[:].unsqueeze(2).broadcast_to((128, 4, H))
            nc.vector.tensor_tensor(o_sb[:, 4 * b:4 * b + 4, :],
                                    o_ps[:, :, 0:H], rec_b,
                                    mybir.AluOpType.mult)
            isl = slice(b * 512, (b + 1) * 512)
            nc.sync.dma_start(
                out_d[isl, :].rearrange("(a p) h -> p a h", p=128),
                o_sb[:, 4 * b:4 * b + 4, :])

        # time-ordered emission: proj chunks interleaved with attention blocks
        emit_proj(0)
        emit_sdiag(0)
        emit_proj(1)
        emit_sfull(1, 0)
        emit_sfull(1, 1)
        for m in range(4):
            emit_pv(0, m)
        emit_sdiag(1)
        emit_proj(2)
        emit_epilogue(0)
        emit_sfull(2, 0)
        emit_pv(1, 0)
        emit_pv(1, 1)
        emit_proj(3)
        emit_sfull(2, 1)
        emit_pv(1, 2)
        emit_pv(1, 3)
        emit_sfull(2, 2)
        emit_sdiag(2)
        emit_epilogue(1)
        emit_sfull(3, 0)
        emit_pv(2, 0)
        emit_pv(2, 1)
        emit_sfull(3, 1)
        emit_pv(2, 2)
        emit_pv(2, 3)
        emit_sfull(3, 2)
        emit_sfull(3, 3)
        emit_sdiag(3, 0)
        emit_epilogue(2)
        emit_pv(3, 0)
        emit_pv(3, 1)
        emit_sdiag(3, 1)
        emit_pv(3, 2)
        emit_pv(3, 3)
        emit_epilogue(3)


def build_nc(repeat=1):
    nc = bacc.Bacc("TRN2", target_bir_lowering=False, debug=False, num_devices=8)
    x_d = nc.dram_tensor("x", [T, D], BF16, kind="ExternalInput")
    w_d = nc.dram_tensor("w", [128, ND, 192], BF16, kind="ExternalInput")
    b_d = nc.dram_tensor("b", [H, 2], F32, kind="ExternalInput")
    out_d = nc.dram_tensor("out", [T, H], BF16, kind="ExternalOutput")
    dram = (x_d, w_d, b_d, out_d)

    from contextlib import ExitStack
    with tile.TileContext(nc) as tc:
        with ExitStack() as ctx:
            build_body(nc, tc, ctx, dram, repeat=repeat)
    nc.compile()
    return nc


_NC_CACHE = {}


def _get_nc(repeat=1):
    if repeat not in _NC_CACHE:
        _NC_CACHE[repeat] = build_nc(repeat)
    return _NC_CACHE[repeat]


def make_in_maps(x, Wk, bk, Wq, bq, Wv, bv):
    scale = float(H) ** -0.5
    bf = ml_dtypes.bfloat16
    w = np.concatenate(
        [Wk.reshape(ND, 128, H), (Wq * scale).reshape(ND, 128, H),
         Wv.reshape(ND, 128, H)], axis=2)          # [ND, 128, 192]
    w = np.ascontiguousarray(w.transpose(1, 0, 2)).astype(bf)  # [128, ND, 192]
    b = np.stack([bq * scale, bv], axis=1).astype(np.float32)
    xb = x.astype(bf)
    return [
        {"x": np.ascontiguousarray(xb[i]), "w": w,
         "b": np.ascontiguousarray(b)}
        for i in range(B)
    ]


def kernel(x, Wk, bk, Wq, bq, Wv, bv, _repeat=1):
    x = np.asarray(x, dtype=np.float32)
    Wk = np.asarray(Wk, dtype=np.float32)
    bk = np.asarray(bk, dtype=np.float32)
    Wq = np.asarray(Wq, dtype=np.float32)
    bq = np.asarray(bq, dtype=np.float32)
    Wv = np.asarray(Wv, dtype=np.float32)
    bv = np.asarray(bv, dtype=np.float32)

    nc = _get_nc(_repeat)
    in_maps = make_in_maps(x, Wk, bk, Wq, bq, Wv, bv)
    res = run_bass_kernel_spmd(nc, in_maps, core_ids=list(range(B)))
    out = np.stack([np.asarray(res.results[i]["out"], dtype=np.float32)
                    for i in range(B)], axis=0)
    return out


# revision 3
# speedup vs baseline: 1.3026x; 1.0679x over previous
"""Single-head causal self-attention (B=8, T=2048, D=512, H=64), data-parallel
over batch across 8 NeuronCores — v9.

vs v8:
  - S^T slots are [128, 1024] pairs in their own pool (psS bufs=2) so the
    slot rotation alternates purely between S/exp: S(k) overlaps exp(k-1)
  - kq projection has its own psK pool (bufs=2): proj pipelines with DMA
  - diag slot A packs m0 [0:512], m1 [512:896], m3 [896:1024] (ap 1024,
    exact trim); slot B holds m2 [0:256]
  - PSUM: psK 2 + psS 4 + psX 1 + psO 1 = 8 banks
"""

import sys

for _p in ("/root/.axon_site/_ro/trn_rl_repo", "/opt/trn_rl_repo"):
    if _p not in sys.path:
        sys.path.append(_p)

import numpy as np
import ml_dtypes
import concourse.bass as bass
import concourse.bacc as bacc
import concourse.tile as tile
from concourse import mybir
from concourse.bass_utils import run_bass_kernel_spmd

F32 = mybir.dt.float32
BF16 = mybir.dt.bfloat16
EXP = mybir.ActivationFunctionType.Exp

B, T, D, H = 8, 2048, 512, 64
ND = D // 128
NT = T // 128
NB = T // 512

# diag piece placement: (slot, offset, n) for m = 0..3
# slot A: m0 [0:512], m1 [512:896], m3 [896:1024]; slot B: m2 [0:256]
DIAG = [(0, 0, 512), (0, 512, 384), (1, 0, 256), (0, 896, 128)]


def build_body(nc, tc, ctx, dram, repeat=1):
    x_d, w_d, out_d = dram

    persist = ctx.enter_context(tc.tile_pool(name="persist", bufs=1))
    epool = ctx.enter_context(tc.tile_pool(name="epool", bufs=12))
    rpool = ctx.enter_context(tc.tile_pool(name="rpool", bufs=2))
    psK = ctx.enter_context(tc.tile_pool(name="psK", bufs=2, space="PSUM"))
    psS = ctx.enter_context(tc.tile_pool(name="psS", bufs=2, space="PSUM"))
    psX = ctx.enter_context(tc.tile_pool(name="psX", bufs=1, space="PSUM"))
    psO = ctx.enter_context(tc.tile_pool(name="psO", bufs=1, space="PSUM"))

    # --- PE warmup through the DMA lead-in ---
    wu = persist.tile([1, 512], BF16)
    nc.vector.memset(wu[:], 0.0)
    for i in range(10):
        wu_ps = psK.tile([16, 512], F32, tag="k", name=f"wu{i}")
        nc.tensor.matmul(wu_ps[:], wu[:, 0:16], wu[:], start=True, stop=True)

    # --- persistent activations ---
    xT = [persist.tile([128, ND, 512], BF16, name=f"xT{t}") for t in range(4)]
    k_sb = persist.tile([H, T], BF16)
    q_sb = persist.tile([H, T], BF16)
    vT = persist.tile([H, T], BF16)
    v_aug = persist.tile([128, NT, H + 1], BF16)
    o_sb = persist.tile([128, NT, H], BF16)

    # --- constants ---
    w_all = persist.tile([128, ND * 192 + 2], BF16)
    b_all = persist.tile([H, 2], F32)

    tri = persist.tile([128, 128], BF16)
    nc.vector.memset(tri[:], 1.0)
    nc.gpsimd.affine_select(out=tri[:], in_=tri[:],
                            compare_op=mybir.AluOpType.is_ge, fill=0.0,
                            base=0, pattern=[[1, 128]], channel_multiplier=-1)

    ident = persist.tile([H, H], BF16)
    nc.vector.memset(ident[:], 1.0)
    nc.gpsimd.affine_select(out=ident[:], in_=ident[:],
                            compare_op=mybir.AluOpType.is_equal, fill=0.0,
                            base=0, pattern=[[1, H]], channel_multiplier=-1)

    nc.vector.memset(v_aug[:, :, H:H + 1], 1.0)

    for _rep in range(repeat):
        if _rep == 0:
            nc.gpsimd.dma_start(w_all[:], w_d[:])
            nc.vector.tensor_copy(b_all[:], w_all[0:H, ND * 192:ND * 192 + 2])
        for tch in range(4):
            tsl = slice(tch * 512, (tch + 1) * 512)
            eng = nc.sync if tch % 2 == 0 else nc.scalar
            eng.dma_start_transpose(xT[tch][:], x_d[tsl, :])

        e_full = {}   # (b, g) -> tile; g = jt//2
        e_diag = {}   # (b, sl) -> tile
        o_ps_all = {}

        def emit_proj_kq(tch):
            tsl = slice(tch * 512, (tch + 1) * 512)
            kq_ps = psK.tile([128, 512], F32, tag="k", name=f"kq{tch}")
            for dc in range(ND):
                nc.tensor.matmul(kq_ps[:], w_all[:, dc * 192:dc * 192 + 128],
                                 xT[tch][:, dc, :],
                                 start=(dc == 0), stop=(dc == ND - 1))
            nc.vector.tensor_scalar_add(q_sb[:, tsl], kq_ps[H:128, :],
                                        b_all[:, 0:1])
            nc.vector.tensor_copy(k_sb[:, tsl], kq_ps[0:H, :])

        def emit_proj_v(tch):
            tsl = slice(tch * 512, (tch + 1) * 512)
            v_ps = psX.tile([H, 512], F32, tag="px")
            for dc in range(ND):
                nc.tensor.matmul(v_ps[:],
                                 w_all[:, dc * 192 + 128:dc * 192 + 192],
                                 xT[tch][:, dc, :],
                                 start=(dc == 0), stop=(dc == ND - 1))
            nc.vector.tensor_scalar_add(vT[:, tsl], v_ps[:], b_all[:, 1:2])
            vt_ps = psX.tile([128, 4, H], BF16, tag="px")
            for q in range(4):
                jt = 4 * tch + q
                nc.tensor.transpose(vt_ps[:, q, :],
                                    vT[:, jt * 128:(jt + 1) * 128], ident[:])
            nc.vector.tensor_copy(v_aug[:, 4 * tch:4 * tch + 4, 0:H], vt_ps[:])

        def emit_spair(b, g):
            isl = slice(b * 512, (b + 1) * 512)
            st = psS.tile([128, 1024], F32, tag="st", name=f"stf{b}_{g}")
            for h in range(2):
                jt = 2 * g + h
                nc.tensor.matmul(st[:, 512 * h:512 * (h + 1)],
                                 k_sb[:, jt * 128:(jt + 1) * 128],
                                 q_sb[:, isl], start=True, stop=True)
            ef = epool.tile([128, 1024], BF16, tag="e", name=f"ef{b}_{g}")
            nc.scalar.activation(ef[:], st[:], EXP)
            e_full[(b, g)] = ef

        def emit_sdiag(b, sl):
            ms = (0, 1, 3) if sl == 0 else (2,)
            st = psS.tile([128, 1024], F32, tag="st", name=f"std{b}_{sl}")
            used = 1024 if sl == 0 else 256
            for m in ms:
                _, off, n = DIAG[m]
                jt = 4 * b + m
                nc.tensor.matmul(
                    st[:, off:off + n],
                    k_sb[:, jt * 128:(jt + 1) * 128],
                    q_sb[:, b * 512 + 128 * m:(b + 1) * 512],
                    start=True, stop=True)
            ed = epool.tile([128, 1024], BF16, tag="e", name=f"ed{b}_{sl}")
            nc.scalar.activation(ed[:, 0:used], st[:, 0:used], EXP)
            for m in ms:
                _, off, _ = DIAG[m]
                nc.vector.tensor_tensor(ed[:, off:off + 128],
                                        ed[:, off:off + 128], tri[:],
                                        mybir.AluOpType.mult)
            e_diag[(b, sl)] = ed

        def emit_pv(b, m_loc):
            if m_loc == 0:
                o_ps_all[b] = psO.tile([128, 4, H + 1], F32, tag="o",
                                       name=f"o_ps{b}")
            o_ps = o_ps_all[b]
            it = 4 * b + m_loc
            order = [jt for jt in range(4 * b)] + \
                    [4 * b + m for m in range(m_loc + 1)]
            for idx, jt in enumerate(order):
                if jt < 4 * b:
                    src = e_full[(b, jt // 2)]
                    col = 512 * (jt % 2) + 128 * m_loc
                else:
                    m = jt - 4 * b
                    sl, off, _ = DIAG[m]
                    src = e_diag[(b, sl)]
                    col = off + 128 * (m_loc - m)
                nc.tensor.matmul(o_ps[:, m_loc, :],
                                 src[:, col:col + 128],
                                 v_aug[:, jt, :],
                                 start=(idx == 0), stop=(idx == len(order) - 1))

        def emit_epilogue(b):
            o_ps = o_ps_all[b]
            rec = rpool.tile([128, 4], F32, tag="r")
            nc.vector.reciprocal(rec[:], o_ps[:, :, H:H + 1].rearrange(
                "p a o -> p (a o)"))
            rec_b = rec[:].unsqueeze(2).broadcast_to((128, 4, H))
            nc.vector.tensor_tensor(o_sb[:, 4 * b:4 * b + 4, :],
                                    o_ps[:, :, 0:H], rec_b,
                                    mybir.AluOpType.mult)
            isl = slice(b * 512, (b + 1) * 512)
            nc.sync.dma_start(
                out_d[isl, :].rearrange("(a p) h -> p a h", p=128),
                o_sb[:, 4 * b:4 * b + 4, :])

        # arrival-aware emission, order forced via tile_wait_until slots
        SCHED = [
            (4.7, lambda: emit_proj_kq(0)),
            (7.1, lambda: emit_proj_kq(2)),
            (7.4, lambda: emit_sdiag(0, 0)),
            (7.6, lambda: emit_sdiag(0, 1)),
            (7.8, lambda: emit_proj_v(0)),
            (9.4, lambda: emit_proj_kq(1)),
            (9.7, lambda: emit_sdiag(2, 0)),
            (9.9, lambda: emit_sdiag(2, 1)),
            (10.0, lambda: emit_spair(2, 0)),
            (10.2, lambda: emit_spair(2, 1)),
            (10.3, lambda: emit_pv(0, 0)),
            (10.35, lambda: emit_pv(0, 1)),
            (10.4, lambda: emit_pv(0, 2)),
            (10.45, lambda: emit_pv(0, 3)),
            (10.5, lambda: emit_proj_v(2)),
            (10.8, lambda: emit_sdiag(1, 0)),
            (11.0, lambda: emit_sdiag(1, 1)),
            (11.1, lambda: emit_spair(1, 0)),
            (11.3, lambda: emit_spair(1, 1)),
            (11.35, lambda: emit_epilogue(0)),
            (11.4, lambda: emit_proj_kq(3)),
            (11.6, lambda: emit_spair(2, 2)),
            (11.8, lambda: emit_spair(2, 3)),
            (11.9, lambda: emit_proj_v(1)),
            (12.3, lambda: emit_sdiag(3, 0)),
            (12.5, lambda: emit_sdiag(3, 1)),
            (12.6, lambda: emit_pv(1, 0)),
            (12.65, lambda: emit_pv(1, 1)),
            (12.7, lambda: emit_pv(1, 2)),
            (12.75, lambda: emit_pv(1, 3)),
            (12.8, lambda: emit_proj_v(3)),
            (12.9, lambda: emit_spair(3, 0)),
            (13.1, lambda: emit_spair(3, 1)),
            (13.15, lambda: emit_epilogue(1)),
            (13.3, lambda: emit_pv(2, 0)),
            (13.35, lambda: emit_pv(2, 1)),
            (13.4, lambda: emit_pv(2, 2)),
            (13.45, lambda: emit_pv(2, 3)),
            (13.6, lambda: emit_spair(3, 2)),
            (13.8, lambda: emit_spair(3, 3)),
            (14.0, lambda: emit_spair(3, 4)),
            (14.2, lambda: emit_spair(3, 5)),
            (14.3, lambda: emit_epilogue(2)),
            (14.5, lambda: emit_pv(3, 0)),
            (14.55, lambda: emit_pv(3, 1)),
            (14.6, lambda: emit_pv(3, 2)),
            (14.65, lambda: emit_pv(3, 3)),
            (14.8, lambda: emit_epilogue(3)),
        ]
        for ts_us, fn in SCHED:
            with tc.tile_wait_until(ts_us * 1e-3):
                fn()

def build_nc(repeat=1):
    nc = bacc.Bacc("TRN2", target_bir_lowering=False, debug=False, num_devices=8)
    x_d = nc.dram_tensor("x", [T, D], BF16, kind="ExternalInput")
    w_d = nc.dram_tensor("w", [128, ND * 192 + 2], BF16, kind="ExternalInput")
    out_d = nc.dram_tensor("out", [T, H], BF16, kind="ExternalOutput")
    dram = (x_d, w_d, out_d)

    from contextlib import ExitStack
    with tile.TileContext(nc) as tc:
        with ExitStack() as ctx:
            build_body(nc, tc, ctx, dram, repeat=repeat)
    nc.compile()
    return nc


_NC_CACHE = {}


def _get_nc(repeat=1):
    if repeat not in _NC_CACHE:
        _NC_CACHE[repeat] = build_nc(repeat)
    return _NC_CACHE[repeat]


def make_in_maps(x, Wk, bk, Wq, bq, Wv, bv):
    scale = float(H) ** -0.5
    bf = ml_dtypes.bfloat16
    w = np.concatenate(
        [Wk.reshape(ND, 128, H), (Wq * scale).reshape(ND, 128, H),
         Wv.reshape(ND, 128, H)], axis=2)
    w = np.ascontiguousarray(w.transpose(1, 0, 2)).reshape(128, ND * 192)
    b = np.zeros((128, 2), dtype=np.float32)
    b[0:H, 0] = bq * scale
    b[0:H, 1] = bv
    w = np.concatenate([w, b], axis=1).astype(bf)
    xb = x.astype(bf)
    return [
        {"x": np.ascontiguousarray(xb[i]), "w": np.ascontiguousarray(w)}
        for i in range(B)
    ]


def kernel(x, Wk, bk, Wq, bq, Wv, bv, _repeat=1):
    x = np.asarray(x, dtype=np.float32)
    Wk = np.asarray(Wk, dtype=np.float32)
    bk = np.asarray(bk, dtype=np.float32)
    Wq = np.asarray(Wq, dtype=np.float32)
    bq = np.asarray(bq, dtype=np.float32)
    Wv = np.asarray(Wv, dtype=np.float32)
    bv = np.asarray(bv, dtype=np.float32)

    nc = _get_nc(_repeat)
    in_maps = make_in_maps(x, Wk, bk, Wq, bq, Wv, bv)
    res = run_bass_kernel_spmd(nc, in_maps, core_ids=list(range(B)))
    out = np.stack([np.asarray(res.results[i]["out"], dtype=np.float32)
                    for i in range(B)], axis=0)
    return out


# revision 4
# speedup vs baseline: 1.3036x; 1.0008x over previous
"""Single-head causal self-attention (B=8, T=2048, D=512, H=64), data-parallel
over batch across 8 NeuronCores — v9.

vs v8:
  - S^T slots are [128, 1024] pairs in their own pool (psS bufs=2) so the
    slot rotation alternates purely between S/exp: S(k) overlaps exp(k-1)
  - kq projection has its own psK pool (bufs=2): proj pipelines with DMA
  - diag slot A packs m0 [0:512], m1 [512:896], m3 [896:1024] (ap 1024,
    exact trim); slot B holds m2 [0:256]
  - PSUM: psK 2 + psS 4 + psX 1 + psO 1 = 8 banks
"""

import sys

for _p in ("/root/.axon_site/_ro/trn_rl_repo", "/opt/trn_rl_repo"):
    if _p not in sys.path:
        sys.path.append(_p)

import numpy as np
import ml_dtypes
import concourse.bass as bass
import concourse.bacc as bacc
import concourse.tile as tile
from concourse import mybir
from concourse.bass_utils import run_bass_kernel_spmd

F32 = mybir.dt.float32
BF16 = mybir.dt.bfloat16
EXP = mybir.ActivationFunctionType.Exp

B, T, D, H = 8, 2048, 512, 64
ND = D // 128
NT = T // 128
NB = T // 512

# diag piece placement: (slot, offset, n) for m = 0..3
# slot A: m0 [0:512], m1 [512:896], m3 [896:1024]; slot B: m2 [0:256]
DIAG = [(0, 0, 512), (0, 512, 384), (1, 0, 256), (0, 896, 128)]


def build_body(nc, tc, ctx, dram, repeat=1):
    x_d, w_d, out_d = dram

    persist = ctx.enter_context(tc.tile_pool(name="persist", bufs=1))
    epool = ctx.enter_context(tc.tile_pool(name="epool", bufs=12))
    rpool = ctx.enter_context(tc.tile_pool(name="rpool", bufs=2))
    psK = ctx.enter_context(tc.tile_pool(name="psK", bufs=2, space="PSUM"))
    psS = ctx.enter_context(tc.tile_pool(name="psS", bufs=2, space="PSUM"))
    psX = ctx.enter_context(tc.tile_pool(name="psX", bufs=1, space="PSUM"))
    psO = ctx.enter_context(tc.tile_pool(name="psO", bufs=1, space="PSUM"))

    # --- PE warmup through the DMA lead-in ---
    wu = persist.tile([1, 512], BF16)
    nc.vector.memset(wu[:], 0.0)
    for i in range(10):
        wu_ps = psK.tile([16, 512], F32, tag="k", name=f"wu{i}")
        nc.tensor.matmul(wu_ps[:], wu[:, 0:16], wu[:], start=True, stop=True)

    # --- persistent activations ---
    xT = [persist.tile([128, ND, 512], BF16, name=f"xT{t}") for t in range(4)]
    k_sb = persist.tile([H, T], BF16)
    q_sb = persist.tile([H, T], BF16)
    vT = persist.tile([H, T], BF16)
    v_aug = persist.tile([128, NT, H + 1], BF16)
    o_sb = persist.tile([128, NT, H], BF16)

    # --- constants ---
    w_all = persist.tile([128, ND * 192 + 2], BF16)
    b_all = persist.tile([H, 2], F32)

    tri = persist.tile([128, 128], BF16)
    nc.vector.memset(tri[:], 1.0)
    nc.gpsimd.affine_select(out=tri[:], in_=tri[:],
                            compare_op=mybir.AluOpType.is_ge, fill=0.0,
                            base=0, pattern=[[1, 128]], channel_multiplier=-1)

    ident = persist.tile([H, H], BF16)
    nc.vector.memset(ident[:], 1.0)
    nc.gpsimd.affine_select(out=ident[:], in_=ident[:],
                            compare_op=mybir.AluOpType.is_equal, fill=0.0,
                            base=0, pattern=[[1, H]], channel_multiplier=-1)

    nc.vector.memset(v_aug[:, :, H:H + 1], 1.0)

    for _rep in range(repeat):
        if _rep == 0:
            nc.gpsimd.dma_start(w_all[:], w_d[:])
            nc.vector.tensor_copy(b_all[:], w_all[0:H, ND * 192:ND * 192 + 2])
        for tch in range(4):
            tsl = slice(tch * 512, (tch + 1) * 512)
            eng = nc.sync if tch % 2 == 0 else nc.scalar
            eng.dma_start_transpose(xT[tch][:], x_d[tsl, :])

        e_full = {}   # (b, g) -> tile; g = jt//2
        e_diag = {}   # (b, sl) -> tile
        o_ps_all = {}

        def emit_proj_kq(tch):
            tsl = slice(tch * 512, (tch + 1) * 512)
            kq_ps = psK.tile([128, 512], F32, tag="k", name=f"kq{tch}")
            for dc in range(ND):
                nc.tensor.matmul(kq_ps[:], w_all[:, dc * 192:dc * 192 + 128],
                                 xT[tch][:, dc, :],
                                 start=(dc == 0), stop=(dc == ND - 1))
            nc.vector.tensor_scalar_add(q_sb[:, tsl], kq_ps[H:128, :],
                                        b_all[:, 0:1])
            nc.vector.tensor_copy(k_sb[:, tsl], kq_ps[0:H, :])

        def emit_proj_v(tch):
            tsl = slice(tch * 512, (tch + 1) * 512)
            v_ps = psX.tile([H, 512], F32, tag="px")
            for dc in range(ND):
                nc.tensor.matmul(v_ps[:],
                                 w_all[:, dc * 192 + 128:dc * 192 + 192],
                                 xT[tch][:, dc, :],
                                 start=(dc == 0), stop=(dc == ND - 1))
            nc.vector.tensor_scalar_add(vT[:, tsl], v_ps[:], b_all[:, 1:2])
            vt_ps = psX.tile([128, 4, H], BF16, tag="px")
            for q in range(4):
                jt = 4 * tch + q
                nc.tensor.transpose(vt_ps[:, q, :],
                                    vT[:, jt * 128:(jt + 1) * 128], ident[:])
            nc.vector.tensor_copy(v_aug[:, 4 * tch:4 * tch + 4, 0:H], vt_ps[:])

        def emit_spair(b, g):
            isl = slice(b * 512, (b + 1) * 512)
            st = psS.tile([128, 1024], F32, tag="st", name=f"stf{b}_{g}")
            for h in range(2):
                jt = 2 * g + h
                nc.tensor.matmul(st[:, 512 * h:512 * (h + 1)],
                                 k_sb[:, jt * 128:(jt + 1) * 128],
                                 q_sb[:, isl], start=True, stop=True)
            ef = epool.tile([128, 1024], BF16, tag="e", name=f"ef{b}_{g}")
            nc.scalar.activation(ef[:], st[:], EXP)
            e_full[(b, g)] = ef

        def emit_sdiag(b, sl):
            ms = (0, 1, 3) if sl == 0 else (2,)
            st = psS.tile([128, 1024], F32, tag="st", name=f"std{b}_{sl}")
            used = 1024 if sl == 0 else 256
            for m in ms:
                _, off, n = DIAG[m]
                jt = 4 * b + m
                nc.tensor.matmul(
                    st[:, off:off + n],
                    k_sb[:, jt * 128:(jt + 1) * 128],
                    q_sb[:, b * 512 + 128 * m:(b + 1) * 512],
                    start=True, stop=True)
            ed = epool.tile([128, 1024], BF16, tag="e", name=f"ed{b}_{sl}")
            nc.scalar.activation(ed[:, 0:used], st[:, 0:used], EXP)
            for m in ms:
                _, off, _ = DIAG[m]
                nc.vector.tensor_tensor(ed[:, off:off + 128],
                                        ed[:, off:off + 128], tri[:],
                                        mybir.AluOpType.mult)
            e_diag[(b, sl)] = ed

        def emit_pv(b, m_loc):
            if m_loc == 0:
                o_ps_all[b] = psO.tile([128, 4, H + 1], F32, tag="o",
                                       name=f"o_ps{b}")
            o_ps = o_ps_all[b]
            it = 4 * b + m_loc
            order = [4 * b + m for m in range(m_loc + 1)] + \
                    [jt for jt in range(4 * b)]
            for idx, jt in enumerate(order):
                if jt < 4 * b:
                    src = e_full[(b, jt // 2)]
                    col = 512 * (jt % 2) + 128 * m_loc
                else:
                    m = jt - 4 * b
                    sl, off, _ = DIAG[m]
                    src = e_diag[(b, sl)]
                    col = off + 128 * (m_loc - m)
                nc.tensor.matmul(o_ps[:, m_loc, :],
                                 src[:, col:col + 128],
                                 v_aug[:, jt, :],
                                 start=(idx == 0), stop=(idx == len(order) - 1))

        def emit_epilogue(b):
            o_ps = o_ps_all[b]
            rec = rpool.tile([128, 4], F32, tag="r")
            nc.vector.reciprocal(rec[:], o_ps[:, :, H:H + 1].rearrange(
                "p a o -> p (a o)"))
            rec_b = rec[:].unsqueeze(2).broadcast_to((128, 4, H))
            nc.vector.tensor_tensor(o_sb[:, 4 * b:4 * b + 4, :],
                                    o_ps[:, :, 0:H], rec_b,
                                    mybir.AluOpType.mult)
            isl = slice(b * 512, (b + 1) * 512)
            nc.sync.dma_start(
                out_d[isl, :].rearrange("(a p) h -> p a h", p=128),
                o_sb[:, 4 * b:4 * b + 4, :])

        # arrival-aware emission, order forced via tile_wait_until slots
        SCHED = [
            (4.7, lambda: emit_proj_kq(0)),
            (7.1, lambda: emit_proj_kq(2)),
            (7.4, lambda: emit_sdiag(0, 0)),
            (7.6, lambda: emit_sdiag(0, 1)),
            (7.8, lambda: emit_proj_v(0)),
            (9.4, lambda: emit_proj_kq(1)),
            (9.7, lambda: emit_sdiag(2, 0)),
            (9.9, lambda: emit_sdiag(2, 1)),
            (10.0, lambda: emit_spair(2, 0)),
            (10.2, lambda: emit_spair(2, 1)),
            (10.3, lambda: emit_pv(0, 0)),
            (10.35, lambda: emit_pv(0, 1)),
            (10.4, lambda: emit_pv(0, 2)),
            (10.45, lambda: emit_pv(0, 3)),
            (10.5, lambda: emit_proj_v(2)),
            (10.8, lambda: emit_sdiag(1, 0)),
            (11.0, lambda: emit_sdiag(1, 1)),
            (11.1, lambda: emit_spair(1, 0)),
            (11.3, lambda: emit_spair(1, 1)),
            (11.35, lambda: emit_epilogue(0)),
            (11.4, lambda: emit_proj_kq(3)),
            (11.6, lambda: emit_spair(2, 2)),
            (11.8, lambda: emit_spair(2, 3)),
            (11.9, lambda: emit_proj_v(1)),
            (12.3, lambda: emit_sdiag(3, 0)),
            (12.5, lambda: emit_sdiag(3, 1)),
            (12.6, lambda: emit_pv(1, 0)),
            (12.65, lambda: emit_pv(1, 1)),
            (12.7, lambda: emit_pv(1, 2)),
            (12.75, lambda: emit_pv(1, 3)),
            (12.8, lambda: emit_proj_v(3)),
            (12.9, lambda: emit_spair(3, 0)),
            (13.1, lambda: emit_spair(3, 1)),
            (13.15, lambda: emit_epilogue(1)),
            (13.3, lambda: emit_pv(2, 0)),
            (13.35, lambda: emit_pv(2, 1)),
            (13.4, lambda: emit_pv(2, 2)),
            (13.45, lambda: emit_pv(2, 3)),
            (13.6, lambda: emit_spair(3, 2)),
            (13.8, lambda: emit_spair(3, 3)),
            (14.0, lambda: emit_spair(3, 4)),
            (14.2, lambda: emit_spair(3, 5)),
            (14.3, lambda: emit_epilogue(2)),
            (14.5, lambda: emit_pv(3, 0)),
            (14.55, lambda: emit_pv(3, 1)),
            (14.6, lambda: emit_pv(3, 2)),
            (14.65, lambda: emit_pv(3, 3)),
            (14.8, lambda: emit_epilogue(3)),
        ]
        for ts_us, fn in SCHED:
            with tc.tile_wait_until(ts_us * 1e-3):
                fn()

def build_nc(repeat=1):
    nc = bacc.Bacc("TRN2", target_bir_lowering=False, debug=False, num_devices=8)
    x_d = nc.dram_tensor("x", [T, D], BF16, kind="ExternalInput")
    w_d = nc.dram_tensor("w", [128, ND * 192 + 2], BF16, kind="ExternalInput")
    out_d = nc.dram_tensor("out", [T, H], BF16, kind="ExternalOutput")
    dram = (x_d, w_d, out_d)

    from contextlib import ExitStack
    with tile.TileContext(nc) as tc:
        with ExitStack() as ctx:
            build_body(nc, tc, ctx, dram, repeat=repeat)
    nc.compile()
    return nc


_NC_CACHE = {}


def _get_nc(repeat=1):
    if repeat not in _NC_CACHE:
        _NC_CACHE[repeat] = build_nc(repeat)
    return _NC_CACHE[repeat]


def make_in_maps(x, Wk, bk, Wq, bq, Wv, bv):
    scale = float(H) ** -0.5
    bf = ml_dtypes.bfloat16
    w = np.concatenate(
        [Wk.reshape(ND, 128, H), (Wq * scale).reshape(ND, 128, H),
         Wv.reshape(ND, 128, H)], axis=2)
    w = np.ascontiguousarray(w.transpose(1, 0, 2)).reshape(128, ND * 192)
    b = np.zeros((128, 2), dtype=np.float32)
    b[0:H, 0] = bq * scale
    b[0:H, 1] = bv
    w = np.concatenate([w, b], axis=1).astype(bf)
    xb = x.astype(bf)
    return [
        {"x": np.ascontiguousarray(xb[i]), "w": np.ascontiguousarray(w)}
        for i in range(B)
    ]


def kernel(x, Wk, bk, Wq, bq, Wv, bv, _repeat=1):
    x = np.asarray(x, dtype=np.float32)
    Wk = np.asarray(Wk, dtype=np.float32)
    bk = np.asarray(bk, dtype=np.float32)
    Wq = np.asarray(Wq, dtype=np.float32)
    bq = np.asarray(bq, dtype=np.float32)
    Wv = np.asarray(Wv, dtype=np.float32)
    bv = np.asarray(bv, dtype=np.float32)

    nc = _get_nc(_repeat)
    in_maps = make_in_maps(x, Wk, bk, Wq, bq, Wv, bv)
    res = run_bass_kernel_spmd(nc, in_maps, core_ids=list(range(B)))
    out = np.stack([np.asarray(res.results[i]["out"], dtype=np.float32)
                    for i in range(B)], axis=0)
    return out
